# revision 4
# baseline (speedup 1.0000x reference)
"""ConvPMF forward on 8 Trainium2 NeuronCores (Bass/Tile).

Per core (data-parallel over the valid (batch, review) pairs):
  1. indirect-DMA gather of word embeddings, one [128 words, 128 dim] tile
     per instruction (the SWDGE per-instruction cost is the kernel's
     critical path; everything else hides under the gather stream)
  2. PE transpose -> rev [dim, words]
  3. reviews processed in PAIRS laid out [128, 2, 260] with 2-column zero
     margins so all 5 conv taps are full-width shifted matmuls (Conv1d SAME
     as 5 PSUM-accumulated matmuls, N=512); 4 pairs run concurrently on the
     tensor engine via column tiling (M=32 each)
  4. max-softmax pool:  max(softmax(fm)) == 1 / sum_w exp(fm - max_w fm)
     -> reduce_max (DVE), Exp with accum_out (ACT), reciprocal (DVE)
Host: shard the ragged review list, combine pooled vectors into item
embeddings, dot with user factors, add bias.
"""
import math

import numpy as np

import concourse.bass as bass
import concourse.mybir as mybir
import concourse.tile as tile
from concourse import bacc
from concourse.bass import IndirectOffsetOnAxis
from concourse.bass_utils import run_bass_kernel_spmd

f32 = mybir.dt.float32
i32 = mybir.dt.int32

N_CORES = 8
D, F, K = 128, 32, 5           # embed dim, factors (conv out channels), window
W = 256                        # words per review
VOCAB = 50000
WP = W + 4                     # padded pair-row width (2 zero cols each side)

_program_cache: dict[int, bass.Bass] = {}


def _build_program(n_pairs: int) -> bass.Bass:
    """SPMD program: `n_pairs` review-pairs per core (2 reviews each).
    Pair q uses tensor-engine column group q%4; every 4 pairs share one
    [128, 512] PSUM fm tile (supergroup)."""
    wt = 4 * n_pairs           # 128-word gather tiles per core
    n_sg = math.ceil(n_pairs / 4)

    nc = bacc.Bacc("TRN2", target_bir_lowering=False, debug=False)
    embed_d = nc.dram_tensor("embed", [VOCAB, D], f32, kind="ExternalInput")
    idx_d = nc.dram_tensor("idx", [128, wt], i32, kind="ExternalInput")
    # consts: identity [128,128] ++ 5 taps of W_k^T [128, 32] each
    cst_d = nc.dram_tensor("cst", [128, 128 + K * F], f32, kind="ExternalInput")
    pooled_d = nc.dram_tensor("pooled", [128, 2 * n_sg], f32,
                              kind="ExternalOutput")

    with tile.TileContext(nc) as tc:
        with tc.tile_pool(name="const", bufs=1) as cpool, \
             tc.tile_pool(name="gat", bufs=4) as gpool, \
             tc.tile_pool(name="rev", bufs=3) as rpool, \
             tc.tile_pool(name="wrk", bufs=2) as wpool, \
             tc.tile_pool(name="psT", bufs=3, space="PSUM") as tppool, \
             tc.tile_pool(name="psF", bufs=2, space="PSUM") as fmpool:
            idx_sb = cpool.tile([128, wt], i32)
            # first chunk small so gather 0 starts ASAP
            c0 = min(8, wt)
            nc.sync.dma_start(idx_sb[:, 0:c0], idx_d[:, 0:c0])
            if wt > c0:
                nc.sync.dma_start(idx_sb[:, c0:wt], idx_d[:, c0:wt])
            cst_sb = cpool.tile([128, 128 + K * F], f32)
            nc.sync.dma_start(cst_sb[:], cst_d[:])
            ident = cst_sb[:, 0:128]
            pooled_sb = cpool.tile([128, 2 * n_sg], f32)

            def wk(k):
                return cst_sb[:, 128 + k * F:128 + (k + 1) * F]

            fm_tiles = []          # (fm_ps, n_pairs_in_tile)
            fm_ps = None
            for q in range(n_pairs):
                j = q % 4                       # tensor-engine column group
                if j == 0:
                    fm_ps = fmpool.tile([128, 2 * W], f32, tag="fm")
                    fm_tiles.append([fm_ps, 0])
                fm_tiles[-1][1] += 1

                g_sb = gpool.tile([128, 4 * D], f32, tag="g")
                for u in range(4):
                    nc.gpsimd.indirect_dma_start(
                        out=g_sb[:, u * D:(u + 1) * D], out_offset=None,
                        in_=embed_d[:],
                        in_offset=IndirectOffsetOnAxis(
                            ap=idx_sb[:, 4 * q + u:4 * q + u + 1], axis=0))

                rev_ps = tppool.tile([128, 4 * D], f32, tag="revps")
                for u in range(4):
                    nc.tensor.transpose(
                        rev_ps[:, u * D:(u + 1) * D],
                        g_sb[:, u * D:(u + 1) * D], ident)

                rev = rpool.tile([128, 2, WP], f32, tag=f"rev{j}")
                eng = nc.vector if q % 2 == 0 else nc.scalar
                if q % 2 == 0:
                    nc.vector.memset(rev[:, :, 0:2], 0.0)
                    nc.vector.memset(rev[:, :, W + 2:WP], 0.0)
                    nc.vector.tensor_copy(
                        rev[:, :, 2:W + 2],
                        rev_ps[:].rearrange("p (r c) -> p r c", c=W))
                else:
                    nc.scalar.memzero(rev[:, :, 0:2])
                    nc.scalar.memzero(rev[:, :, W + 2:WP])
                    nc.scalar.copy(
                        rev[:, :, 2:W + 2],
                        rev_ps[:].rearrange("p (r c) -> p r c", c=W))

                for ki, k in enumerate(range(K)):
                    nc.tensor.matmul(
                        fm_ps[32 * j:32 * j + 32, :],
                        lhsT=wk(k), rhs=rev[:, :, k:k + W],
                        start=(ki == 0), stop=(ki == K - 1),
                        tile_position=(0, 32 * j))

                if j == 3 or q == n_pairs - 1:
                    sg = len(fm_tiles) - 1
                    npair = fm_tiles[-1][1]
                    rows = 32 * npair
                    negm = wpool.tile([128, 2], f32, tag="negm")
                    nc.vector.tensor_reduce(
                        negm[0:rows, :],
                        fm_ps[0:rows, :].rearrange("p (r c) -> p r c", c=W),
                        axis=mybir.AxisListType.X,
                        op=mybir.AluOpType.max, negate=True)
                    ssum = wpool.tile([128, 2], f32, tag="ssum")
                    e_sb = wpool.tile([128, W], f32, tag="e")
                    for r in range(2):
                        nc.scalar.activation(
                            e_sb[0:rows, :],
                            fm_ps[0:rows, :].rearrange(
                                "p (r c) -> p r c", c=W)[:, r, :],
                            mybir.ActivationFunctionType.Exp,
                            bias=negm[0:rows, r:r + 1], scale=1.0,
                            accum_out=ssum[0:rows, r:r + 1])
                    nc.vector.reciprocal(
                        pooled_sb[0:rows, 2 * sg:2 * sg + 2], ssum[0:rows, :])

            nc.sync.dma_start(pooled_d[:], pooled_sb[:])
    nc.compile()
    return nc


def prepare(user_indices, docs, review_counts, w_user, embed_matrix,
            conv_weight, bias):
    """Host-side sharding prep: returns (nc, in_maps, valid, n_core) or None
    when there are no valid reviews."""
    docs = np.asarray(docs)
    review_counts = np.asarray(review_counts)
    embed_matrix = np.ascontiguousarray(np.asarray(embed_matrix, dtype=np.float32))
    conv_weight = np.asarray(conv_weight, dtype=np.float32)

    b_sz = docs.shape[0]
    valid = [(b, r) for b in range(b_sz) for r in range(int(review_counts[b]))]
    if not valid:
        return None

    n_core = 2 * math.ceil(math.ceil(len(valid) / N_CORES) / 2)
    n_pairs = n_core // 2

    cst = np.zeros((128, 128 + K * F), dtype=np.float32)
    cst[:, 0:128] = np.eye(128, dtype=np.float32)
    for k in range(K):
        cst[:, 128 + k * F:128 + (k + 1) * F] = conv_weight[:, :, k].T

    docs32 = docs.astype(np.int32)
    in_maps = []
    for c in range(N_CORES):
        idx = np.zeros((128, 4 * n_pairs), dtype=np.int32)
        for slot in range(n_core):
            i = c * n_core + slot
            if i >= len(valid):
                break
            bb, rr = valid[i]
            wrds = docs32[bb, rr]                      # [256]
            q, r = slot // 2, slot % 2
            idx[:, 4 * q + 2 * r] = wrds[0:128]
            idx[:, 4 * q + 2 * r + 1] = wrds[128:256]
        in_maps.append({"embed": embed_matrix, "idx": idx, "cst": cst})

    nc = _program_cache.get(n_pairs)
    if nc is None:
        nc = _build_program(n_pairs)
        _program_cache[n_pairs] = nc
    return nc, in_maps, valid, n_core


def kernel(user_indices, docs, review_counts, w_user, embed_matrix, conv_weight,
           bias):
    user_indices = np.asarray(user_indices)
    docs = np.asarray(docs)
    review_counts = np.asarray(review_counts)
    w_user = np.asarray(w_user, dtype=np.float32)
    conv_weight = np.asarray(conv_weight, dtype=np.float32)
    bias = np.asarray(bias, dtype=np.float32)

    b_sz = docs.shape[0]
    denom = np.maximum(review_counts, 1).astype(np.float32)
    prep = prepare(user_indices, docs, review_counts, w_user, embed_matrix,
                   conv_weight, bias)
    if prep is None:
        return np.full((b_sz,), bias[0], dtype=np.float32)
    nc, in_maps, valid, n_core = prep

    res = run_bass_kernel_spmd(nc, in_maps, list(range(N_CORES)))

    item = np.zeros((b_sz, F), dtype=np.float32)
    for i, (bb, rr) in enumerate(valid):
        c, slot = i // n_core, i % n_core
        q, r = slot // 2, slot % 2
        sg, j = q // 4, q % 4
        item[bb] += res.results[c]["pooled"][32 * j:32 * j + 32, 2 * sg + r]
    item /= denom[:, None]
    out = (w_user[user_indices] * item).sum(axis=-1) + bias[0]
    return out.astype(np.float32)
